# revision 38
# baseline (speedup 1.0000x reference)
"""Trainium2 Bass kernel for fp8 quantize-dequantize DenseGeneral + gelu.

Computes: out = gelu(qdq_e4m3fn(x) @ qdq_e4m3fn(W) + round_bf16(bias))
with delayed-scaling fp8 quantization (scale = amax/448 over full tensor,
folded with the amax history), reproducing reference.py numerics.

Distribution (8 NeuronCores, tensor-parallel on F):
  - The host pre-transposes x to x^T and replicates it to every core's
    DRAM in fp16, laid out chunk-major ([64, 128, 32, 128]: 1 MB fully
    contiguous per 128-token chunk) so the matmul lhsT stream runs at
    DMA line rate. Each core computes out[:, its 2048 f-columns] for ALL
    tokens, quantizing x^T chunks on the fly. No AllGather, no PE
    transposes, no runtime-offset DMAs.
  - Amaxes are reduced from the fp16 copies (each core: a disjoint 1/8
    chunk-slice of x^T and its W shard); one 2-float AllReduce(max)
    produces both global amaxes. The first 6 W pairs are parked in SBUF
    during the amax pass so quantization starts the moment the
    AllReduce lands; the rest re-stream.
  - fp16 transport shifts ~0.3% of fp8 rounding decisions and the scale
    by <=2^-11 vs the fp32 reference (measured offline: rel err 1.2e-2
    vs the 2e-2 gate).

fp8 trick: TRN's float8e4 has max 240 (OCP e4m3fn has 448). We store q/2:
multiplying by a power of two preserves round-to-nearest decisions on the
3-bit-mantissa grid, so RNE(v/2) in TRN-fp8 == RNE(v)/2 in e4m3fn for all
|v| >= 2^-5 (below that, absolute error <= 2^-9 * scale - negligible).
The factor 4 is folded into the output scale C = 4 * s_x * s_w.
The matmul runs in fp8 DoubleRow mode (2 fp8 MACs/cell/cycle).
"""

import sys

sys.path.insert(0, "/opt/trn_rl_repo")

import numpy as np
from contextlib import ExitStack

import concourse.bass as bass
import concourse.mybir as mybir
import concourse.tile as tile
from concourse import bacc, bass_isa
from concourse.bass_utils import run_bass_kernel_spmd
from concourse.bass_interp import get_hw_module
from concourse.masks import make_identity

F32 = mybir.dt.float32
F16 = mybir.dt.float16
BF16 = mybir.dt.bfloat16
FP8 = mybir.dt.float8e4
AX = mybir.AxisListType
ALU = mybir.AluOpType
ACTF = mybir.ActivationFunctionType
DR = mybir.MatmulPerfMode.DoubleRow

# Problem shapes (hardcoded per contract)
B, S, D, F = 4, 2048, 4096, 16384
T = B * S
NCORES = 8
HL = 16
E4M3_MAX = 448.0
P = 128
MS = 128                 # tokens per lhsT chunk
WPARK = 6                # W pairs parked in SBUF during the amax pass


def build_program(d, f_shard, t_total, n_cores, hl=HL, act_fn=ACTF.Gelu_apprx_tanh):
    """Build the SPMD per-core bass program. Same program on every core;
    per-core behavior differs only through the input shards."""
    d_tiles = d // P             # 32 contraction subtiles
    n_pairs = d_tiles // 2       # 16 DoubleRow k-pairs
    NF = 512                     # psum free dim
    n_tiles = f_shard // NF      # 4
    n_chunks = t_total // MS     # 64
    loc_chunks = n_chunks // n_cores  # 8 chunks in this core's amax slice

    nc = bacc.Bacc(
        "TRN2",
        target_bir_lowering=False,
        debug=False,
        num_devices=n_cores,
    )

    xt16 = nc.dram_tensor("xt16", [n_chunks, P, d_tiles, MS], F16, kind="ExternalInput")
    x16l = nc.dram_tensor("x16_loc", [loc_chunks, P, d_tiles, MS], F16, kind="ExternalInput")
    w16 = nc.dram_tensor("w16", [n_pairs, P, 2, f_shard], F16, kind="ExternalInput")
    b_sh = nc.dram_tensor("bias_shard", [1, f_shard], F32, kind="ExternalInput")
    ih = nc.dram_tensor("in_hist", [1, hl], F32, kind="ExternalInput")
    kh = nc.dram_tensor("k_hist", [1, hl], F32, kind="ExternalInput")
    out_sh = nc.dram_tensor("out_shard", [t_total, f_shard], F32, kind="ExternalOutput")

    rg = [list(range(n_cores))]
    shared = "Shared" if n_cores > 4 else "Local"

    with tile.TileContext(nc) as tc, ExitStack() as ctx:
        const = ctx.enter_context(tc.tile_pool(name="const", bufs=1))
        small = ctx.enter_context(tc.tile_pool(name="small", bufs=1))
        wpark = ctx.enter_context(tc.tile_pool(name="wpark", bufs=1))
        wsr = ctx.enter_context(tc.tile_pool(name="wsr", bufs=3))
        qwp = ctx.enter_context(tc.tile_pool(name="qw", bufs=1))
        lfp = ctx.enter_context(tc.tile_pool(name="lfp", bufs=3))
        qlp = ctx.enter_context(tc.tile_pool(name="qlp", bufs=3))
        ostg = ctx.enter_context(tc.tile_pool(name="ostg", bufs=3))
        psum = ctx.enter_context(tc.tile_pool(name="psum", bufs=8, space="PSUM"))
        dram = ctx.enter_context(tc.tile_pool(name="dram", bufs=1, space="DRAM"))

        # DMA-trigger queues for the bulk prologue streams (HWDGE only;
        # the gpsimd engine is kept clear to run half the amax reduces).
        queues = [nc.sync, nc.scalar]

        # ---- constants ----
        zbias = const.tile([P, 1], F32)
        nc.gpsimd.memset(zbias[:], 0.0)

        histx = small.tile([1, hl], F32)
        nc.gpsimd.dma_start(histx[:], ih[:])
        histw = small.tile([1, hl], F32)
        nc.gpsimd.dma_start(histw[:], kh[:])

        # ---- phase 1+2: local abs-max, split AllReduces, scales ----
        # The x amax (8 chunks, ~35us of DVE) completes first and its
        # AllReduce + scale chain + chunk-0/1 quantization all hide
        # behind the w amax reduces (16 pairs, ~71us of DVE). AR_w is
        # the only exposed collective. All reduces on the DVE
        # (tensor_reduce only has a 1x uop: ~4.4us/tile).
        # First WPARK w pairs land in park tiles and skip the re-read.
        xacc = small.tile([P, loc_chunks], F32)
        wacc = small.tile([P, n_pairs], F32)
        wp_tiles = [
            wpark.tile([P, 2, f_shard], F16, name=f"wp{k}") for k in range(WPARK)
        ]

        def amax_bounce(acc, sfx):
            # partition reduction via a DRAM bounce (tiny DMAs on the
            # idle gpsimd queue) instead of the ~15us partition_all_reduce
            mp = small.tile([P, 1], F32, name=f"mp_{sfx}")
            nc.vector.reduce_max(mp[:], acc[:], axis=AX.X)
            mpd = dram.tile([P, 1], F32, name=f"mpd_{sfx}")
            nc.gpsimd.dma_start(mpd[:], mp[:])
            mpr = small.tile([1, 1, P], F32, name=f"mpr_{sfx}")
            nc.gpsimd.dma_start(
                mpr[:], mpd[:].rearrange("(o p) c -> o c p", o=1, p=P)
            )
            return mpr

        def amax_allreduce(mpr, sfx):
            ma = small.tile([1, 1], F32, name=f"ma_{sfx}")
            nc.vector.reduce_max(ma[:], mpr[:], axis=AX.X)
            ar_in = dram.tile([1, 1], F32, name=f"arin_{sfx}")
            nc.gpsimd.dma_start(ar_in[:], ma[:])
            ar_out = dram.tile([1, 1], F32, addr_space=shared, name=f"arout_{sfx}")
            nc.gpsimd.collective_compute(
                "AllReduce",
                ALU.max,
                replica_groups=rg,
                ins=[ar_in[:].opt()],
                outs=[ar_out[:].opt()],
            )
            g = small.tile([1, 1], F32, name=f"g_{sfx}")
            nc.gpsimd.dma_start(g[:], ar_out[:])
            return g

        # reference: hist' = [amax_now, hist[0:HL-1]]; amax = max(hist')
        #            sf = 448/amax ; s = 1/sf (dequant scale)
        # ours:      r_half = 0.5*sf (quant multiplier, half-scale trick)
        #            C = 4 * s_x * s_w (output scale)
        def scales(gm, hist, sfx):
            hmx = small.tile([1, 1], F32, name=f"hmx_{sfx}")
            nc.vector.reduce_max(hmx[:], hist[:, 0 : hl - 1], axis=AX.X)
            amax = small.tile([1, 1], F32, name=f"amax_{sfx}")
            nc.vector.tensor_tensor(amax[:], gm, hmx[:], op=ALU.max)
            ra = small.tile([1, 1], F32, name=f"ra_{sfx}")
            nc.vector.reciprocal(ra[:], amax[:])
            sf = small.tile([1, 1], F32, name=f"sf_{sfx}")
            nc.vector.tensor_scalar_mul(sf[:], ra[:], E4M3_MAX)
            s = small.tile([1, 1], F32, name=f"s_{sfx}")
            nc.vector.reciprocal(s[:], sf[:])
            rh = small.tile([1, 1], F32, name=f"rh_{sfx}")
            nc.vector.tensor_scalar_mul(rh[:], sf[:], 0.5)
            return s, rh

        # x amax stream first
        qi = 0
        for i in range(loc_chunks):
            xt = lfp.tile([P, d_tiles, MS], F16, name="lf")
            queues[qi % 2].dma_start(xt[:], x16l[i])
            qi += 1
            nc.vector.reduce_max(
                xacc[:, i : i + 1], xt[:], axis=AX.XY,
                apply_absolute_value=True,
            )
        mpr_x = amax_bounce(xacc, "x")

        def w_amax(k):
            nonlocal qi
            if k < WPARK:
                wt = wp_tiles[k]
            else:
                wt = wsr.tile([P, 2, f_shard], F16, name="wtr")
            queues[qi % 2].dma_start(wt[:], w16[k])
            qi += 1
            nc.vector.reduce_max(
                wacc[:, k : k + 1], wt[:], axis=AX.XY,
                apply_absolute_value=True,
            )

        # w amax stream interleaves with the x-side AR: the bounce lands
        # during w0/w1's reduces, the AR runs during w2..w9, and the x
        # scale chain is reached only after w9 so the DVE never stalls
        # on AR_x latency.
        W_SPLIT = 10
        for k in range(2):
            w_amax(k)
        g_x = amax_allreduce(mpr_x, "x")
        for k in range(2, W_SPLIT):
            w_amax(k)

        # x scale chain lands right as AR_x completes; the chunk-0/1
        # quantization (ScalarE) runs while the DVE finishes the w amax
        s_x, rh_x = scales(g_x[:], histx, "x")
        rhx_b = small.tile([P, 1], F32)
        nc.gpsimd.partition_broadcast(rhx_b[:], rh_x[:])

        # lhsT chunk loader: 1 MB contiguous fp16 DMA + quantize on ScalarE.
        # splits > 1 chop the quant so the first matmuls start sooner.
        def load_chunk(q, splits=1):
            lf = lfp.tile([P, d_tiles, MS], F16, name="lf")
            nc.sync.dma_start(lf[:], xt16[q])
            ql = qlp.tile([P, d_tiles, MS], FP8, name="ql")
            step = d_tiles // splits
            for j in range(splits):
                nc.scalar.mul(
                    ql[:, j * step : (j + 1) * step, :],
                    lf[:, j * step : (j + 1) * step, :],
                    rhx_b[:],
                )
            return ql

        pre = {q: load_chunk(q, splits=4) for q in (0, 1)}

        # w amax stream, rest
        for k in range(W_SPLIT, n_pairs):
            w_amax(k)
        g_w = amax_allreduce(amax_bounce(wacc, "w"), "w")
        s_w, rh_w = scales(g_w[:], histw, "w")
        rhw_b = small.tile([P, 1], F32)
        nc.gpsimd.partition_broadcast(rhw_b[:], rh_w[:])

        Cs = small.tile([1, 1], F32)
        nc.vector.tensor_tensor(Cs[:], s_x[:], s_w[:], op=ALU.mult)
        nc.vector.tensor_scalar_mul(Cs[:], Cs[:], 4.0)
        rC = small.tile([1, 1], F32)
        nc.vector.reciprocal(rC[:], Cs[:])
        C_b = small.tile([P, 1], F32)
        nc.gpsimd.partition_broadcast(C_b[:], Cs[:])

        # bias: fp32 -> bf16 -> fp32, then pre-divide by C, broadcast to 128 parts
        btmp = small.tile([1, f_shard], F32)
        nc.gpsimd.dma_start(btmp[:], b_sh[:])
        bbf = small.tile([1, f_shard], BF16)
        nc.vector.tensor_copy(bbf[:], btmp[:])
        nc.vector.tensor_copy(btmp[:], bbf[:])
        nc.vector.tensor_scalar_mul(btmp[:], btmp[:], rC[:])
        bP = small.tile([P, f_shard], F32)
        nc.gpsimd.partition_broadcast(bP[:], btmp[:])

        # ---- phase 3: quantize w (parked pairs instantly, rest re-stream) ----
        # Quant muls alternate ScalarE/VectorE so the matmul k-loop chases
        # at ~1.5us/pair instead of 3us.
        qw_tiles = [
            qwp.tile([P, 2, f_shard], FP8, name=f"qwt{k}") for k in range(n_pairs)
        ]
        for k in range(n_pairs):
            if k < WPARK:
                wt = wp_tiles[k]
            else:
                wt = wsr.tile([P, 2, f_shard], F16, name="wtr")
                (nc.scalar if k % 2 == 0 else nc.gpsimd).dma_start(wt[:], w16[k])
            if k % 2 == 0:
                nc.scalar.mul(qw_tiles[k][:], wt[:], rhw_b[:])
            else:
                nc.vector.tensor_scalar_mul(qw_tiles[k][:], wt[:], rhw_b[:])

        # ---- phase 4: matmul + epilogue ----
        # out[tok, f] = gelu(C * (sum_d qxT[d, tok] * qw[d, f] + bias/C))
        for q in range(n_chunks):
            ql = pre.pop(q) if q in pre else load_chunk(q)
            pss = [
                psum.tile([P, NF], F32, tag="ps", name=f"mmps{n}")
                for n in range(n_tiles)
            ]
            for k in range(n_pairs):
                for n in range(n_tiles):
                    nc.tensor.matmul(
                        pss[n][:],
                        lhsT=ql[:, 2 * k : 2 * k + 2, :],
                        rhs=qw_tiles[k][:, :, n * NF : (n + 1) * NF],
                        start=(k == 0),
                        stop=(k == n_pairs - 1),
                        perf_mode=DR,
                    )
            row = q * MS
            for n in range(n_tiles):
                t1 = ostg.tile([P, NF], F32, name="t1")
                nc.vector.tensor_tensor(
                    t1[:], pss[n][:], bP[:, n * NF : (n + 1) * NF], op=ALU.add
                )
                ot = ostg.tile([P, NF], F32, name="ot")
                nc.scalar.activation(
                    ot[:], t1[:], act_fn, bias=zbias[:], scale=C_b[:]
                )
                oq = nc.gpsimd if n % 2 == 0 else nc.sync
                oq.dma_start(
                    out_sh[row : row + P, n * NF : (n + 1) * NF], ot[:]
                )

    nc.compile()
    return nc


_CACHE = {}


def _get_program(d=D, f_shard=F // NCORES, t_total=T, n_cores=NCORES):
    key = (d, f_shard, t_total, n_cores)
    if key not in _CACHE:
        _CACHE[key] = build_program(d, f_shard, t_total, n_cores)
    return _CACHE[key]


def make_in_maps(x, w, bias, in_hist, k_hist, n_cores=NCORES):
    t_total = x.shape[0]
    d = x.shape[1]
    f_shard = w.shape[1] // n_cores
    d_tiles = d // P
    n_pairs = d_tiles // 2
    n_chunks = t_total // MS
    loc_chunks = n_chunks // n_cores

    # chunk-major fp16 x^T: L[q, p, s, m] = x[q*MS + m, s*P + p]
    x16 = x.astype(np.float16).reshape(n_chunks, MS, d_tiles, P)
    xt16 = np.ascontiguousarray(x16.transpose(0, 3, 2, 1))  # [64, 128, 32, 128]

    ih = np.asarray(in_hist, np.float32).reshape(1, HL)
    kh = np.asarray(k_hist, np.float32).reshape(1, HL)
    in_maps = []
    for r in range(n_cores):
        # pair-major fp16 W: w16[k, p, o, f] = w[(2k+o)*P + p, f]
        w16 = np.ascontiguousarray(
            w[:, r * f_shard : (r + 1) * f_shard]
            .astype(np.float16)
            .reshape(n_pairs, 2, P, f_shard)
            .transpose(0, 2, 1, 3)
        )
        in_maps.append(
            {
                "xt16": xt16,
                "x16_loc": np.ascontiguousarray(
                    xt16[r * loc_chunks : (r + 1) * loc_chunks]
                ),
                "w16": w16,
                "bias_shard": np.ascontiguousarray(
                    bias[r * f_shard : (r + 1) * f_shard], dtype=np.float32
                ).reshape(1, f_shard),
                "in_hist": ih,
                "k_hist": kh,
            }
        )
    return in_maps


def _install_ntff_shim():
    """Provide antenv.axon_hooks (absent in this image) so bass_utils can
    NTFF-profile under axon, wiring it to libaxon_pjrt's nrt profile API."""
    import sys as _sys
    import types

    if "antenv.axon_hooks" in _sys.modules:
        return
    mod = types.ModuleType("antenv.axon_hooks")
    _state = {"hook": None}
    mod.set_axon_ntff_profile_hook = lambda h: _state.__setitem__("hook", h)
    mod.get_axon_ntff_profile_hook = lambda: _state["hook"]
    _sys.modules["antenv.axon_hooks"] = mod
    import antenv

    antenv.axon_hooks = mod
    try:
        from trn_agent_boot.trn_boot import _ntff_profile_via_ctypes

        mod.set_axon_ntff_profile_hook(
            _ntff_profile_via_ctypes("/opt/axon/libaxon_pjrt.so")
        )
    except Exception as e:
        print(f"ntff shim: hook unavailable ({e}); tracing will degrade")


def run(inputs_dict, trace=False, **kw):
    """Run on the 8 NeuronCores; returns (full_output, BassKernelResults)."""
    if trace:
        _install_ntff_shim()
    x = np.asarray(inputs_dict["inputs"], np.float32).reshape(T, D)
    w = np.asarray(inputs_dict["kernel"], np.float32)
    bias = np.asarray(inputs_dict["bias"], np.float32)
    nc = _get_program()
    in_maps = make_in_maps(
        x, w, bias, inputs_dict["input_amax_history"], inputs_dict["kernel_amax_history"]
    )
    old_m = nc.m
    nc.m = get_hw_module(nc.m)
    try:
        res = run_bass_kernel_spmd(
            nc, in_maps, core_ids=list(range(NCORES)), trace=trace, **kw
        )
    finally:
        nc.m = old_m
    out = np.concatenate(
        [res.results[r]["out_shard"] for r in range(NCORES)], axis=1
    )
    return out.reshape(B, S, F).astype(np.float32), res


def kernel(**inputs):
    out, _ = run(inputs, trace=False)
    return out


# revision 40
# speedup vs baseline: 1.0316x; 1.0316x over previous
"""Trainium2 Bass kernel for fp8 quantize-dequantize DenseGeneral + gelu.

Computes: out = gelu(qdq_e4m3fn(x) @ qdq_e4m3fn(W) + round_bf16(bias))
with delayed-scaling fp8 quantization (scale = amax/448 over full tensor,
folded with the amax history), reproducing reference.py numerics.

Distribution (8 NeuronCores, tensor-parallel on F):
  - The host pre-transposes x to x^T and replicates it to every core's
    DRAM in fp16, laid out chunk-major ([64, 128, 32, 128]: 1 MB fully
    contiguous per 128-token chunk) so the matmul lhsT stream runs at
    DMA line rate. Each core computes out[:, its 2048 f-columns] for ALL
    tokens, quantizing x^T chunks on the fly. No AllGather, no PE
    transposes, no runtime-offset DMAs.
  - Amaxes are reduced from the fp16 copies (each core: a disjoint 1/8
    chunk-slice of x^T and its W shard); one 2-float AllReduce(max)
    produces both global amaxes. The first 6 W pairs are parked in SBUF
    during the amax pass so quantization starts the moment the
    AllReduce lands; the rest re-stream.
  - fp16 transport shifts ~0.3% of fp8 rounding decisions and the scale
    by <=2^-11 vs the fp32 reference (measured offline: rel err 1.2e-2
    vs the 2e-2 gate).

fp8 trick: TRN's float8e4 has max 240 (OCP e4m3fn has 448). We store q/2:
multiplying by a power of two preserves round-to-nearest decisions on the
3-bit-mantissa grid, so RNE(v/2) in TRN-fp8 == RNE(v)/2 in e4m3fn for all
|v| >= 2^-5 (below that, absolute error <= 2^-9 * scale - negligible).
The factor 4 is folded into the output scale C = 4 * s_x * s_w.
The matmul runs in fp8 DoubleRow mode (2 fp8 MACs/cell/cycle).
"""

import sys

sys.path.insert(0, "/opt/trn_rl_repo")

import numpy as np
from contextlib import ExitStack

import concourse.bass as bass
import concourse.mybir as mybir
import concourse.tile as tile
from concourse import bacc, bass_isa
from concourse.bass_utils import run_bass_kernel_spmd
from concourse.bass_interp import get_hw_module
from concourse.masks import make_identity

F32 = mybir.dt.float32
F16 = mybir.dt.float16
BF16 = mybir.dt.bfloat16
FP8 = mybir.dt.float8e4
AX = mybir.AxisListType
ALU = mybir.AluOpType
ACTF = mybir.ActivationFunctionType
DR = mybir.MatmulPerfMode.DoubleRow

# Problem shapes (hardcoded per contract)
B, S, D, F = 4, 2048, 4096, 16384
T = B * S
NCORES = 8
HL = 16
E4M3_MAX = 448.0
P = 128
MS = 128                 # tokens per lhsT chunk
WPARK = 6                # W pairs parked in SBUF during the amax pass


def build_program(d, f_shard, t_total, n_cores, hl=HL, act_fn=ACTF.Gelu_apprx_tanh):
    """Build the SPMD per-core bass program. Same program on every core;
    per-core behavior differs only through the input shards."""
    d_tiles = d // P             # 32 contraction subtiles
    n_pairs = d_tiles // 2       # 16 DoubleRow k-pairs
    NF = 512                     # psum free dim
    n_tiles = f_shard // NF      # 4
    n_chunks = t_total // MS     # 64
    loc_chunks = n_chunks // n_cores  # 8 chunks in this core's amax slice

    nc = bacc.Bacc(
        "TRN2",
        target_bir_lowering=False,
        debug=False,
        num_devices=n_cores,
    )

    xt16 = nc.dram_tensor("xt16", [n_chunks, P, d_tiles, MS], F16, kind="ExternalInput")
    x16l = nc.dram_tensor("x16_loc", [loc_chunks, P, d_tiles, MS], F16, kind="ExternalInput")
    w16 = nc.dram_tensor("w16", [n_pairs, P, 2, f_shard], F16, kind="ExternalInput")
    b_sh = nc.dram_tensor("bias_shard", [1, f_shard], F32, kind="ExternalInput")
    ih = nc.dram_tensor("in_hist", [1, hl], F32, kind="ExternalInput")
    kh = nc.dram_tensor("k_hist", [1, hl], F32, kind="ExternalInput")
    out_sh = nc.dram_tensor("out_shard", [t_total, f_shard], F32, kind="ExternalOutput")

    rg = [list(range(n_cores))]
    shared = "Shared" if n_cores > 4 else "Local"

    with tile.TileContext(nc) as tc, ExitStack() as ctx:
        const = ctx.enter_context(tc.tile_pool(name="const", bufs=1))
        small = ctx.enter_context(tc.tile_pool(name="small", bufs=1))
        wpark = ctx.enter_context(tc.tile_pool(name="wpark", bufs=1))
        wsr = ctx.enter_context(tc.tile_pool(name="wsr", bufs=3))
        qwp = ctx.enter_context(tc.tile_pool(name="qw", bufs=1))
        lfp = ctx.enter_context(tc.tile_pool(name="lfp", bufs=3))
        qlp = ctx.enter_context(tc.tile_pool(name="qlp", bufs=3))
        ostg = ctx.enter_context(tc.tile_pool(name="ostg", bufs=3))
        psum = ctx.enter_context(tc.tile_pool(name="psum", bufs=8, space="PSUM"))
        dram = ctx.enter_context(tc.tile_pool(name="dram", bufs=1, space="DRAM"))

        # DMA-trigger queues for the bulk prologue streams (HWDGE only;
        # the gpsimd engine is kept clear to run half the amax reduces).
        queues = [nc.sync, nc.scalar]

        # ---- constants ----
        zbias = const.tile([P, 1], F32)
        nc.gpsimd.memset(zbias[:], 0.0)

        histx = small.tile([1, hl], F32)
        nc.gpsimd.dma_start(histx[:], ih[:])
        histw = small.tile([1, hl], F32)
        nc.gpsimd.dma_start(histw[:], kh[:])

        # ---- phase 1+2: local abs-max, split AllReduces, scales ----
        # The x amax (8 chunks, ~35us of DVE) completes first and its
        # AllReduce + scale chain + chunk-0/1 quantization all hide
        # behind the w amax reduces (16 pairs, ~71us of DVE). AR_w is
        # the only exposed collective. All reduces on the DVE
        # (tensor_reduce only has a 1x uop: ~4.4us/tile).
        # First WPARK w pairs land in park tiles and skip the re-read.
        xacc = small.tile([P, loc_chunks], F32)
        wacc = small.tile([P, n_pairs], F32)
        wp_tiles = [
            wpark.tile([P, 2, f_shard], F16, name=f"wp{k}") for k in range(WPARK)
        ]

        def amax_bounce(acc, sfx):
            # partition reduction via a DRAM bounce (tiny DMAs on the
            # idle gpsimd queue) instead of the ~15us partition_all_reduce
            mp = small.tile([P, 1], F32, name=f"mp_{sfx}")
            nc.vector.reduce_max(mp[:], acc[:], axis=AX.X)
            mpd = dram.tile([P, 1], F32, name=f"mpd_{sfx}")
            nc.gpsimd.dma_start(mpd[:], mp[:])
            mpr = small.tile([1, 1, P], F32, name=f"mpr_{sfx}")
            nc.gpsimd.dma_start(
                mpr[:], mpd[:].rearrange("(o p) c -> o c p", o=1, p=P)
            )
            return mpr

        def amax_allreduce(mpr, sfx):
            ma = small.tile([1, 1], F32, name=f"ma_{sfx}")
            nc.vector.reduce_max(ma[:], mpr[:], axis=AX.X)
            ar_in = dram.tile([1, 1], F32, name=f"arin_{sfx}")
            nc.gpsimd.dma_start(ar_in[:], ma[:])
            ar_out = dram.tile([1, 1], F32, addr_space=shared, name=f"arout_{sfx}")
            nc.gpsimd.collective_compute(
                "AllReduce",
                ALU.max,
                replica_groups=rg,
                ins=[ar_in[:].opt()],
                outs=[ar_out[:].opt()],
            )
            g = small.tile([1, 1], F32, name=f"g_{sfx}")
            nc.gpsimd.dma_start(g[:], ar_out[:])
            return g

        # reference: hist' = [amax_now, hist[0:HL-1]]; amax = max(hist')
        #            sf = 448/amax ; s = 1/sf (dequant scale)
        # ours:      r_half = 0.5*sf (quant multiplier, half-scale trick)
        #            C = 4 * s_x * s_w (output scale)
        def scales(gm, hist, sfx):
            hmx = small.tile([1, 1], F32, name=f"hmx_{sfx}")
            nc.vector.reduce_max(hmx[:], hist[:, 0 : hl - 1], axis=AX.X)
            amax = small.tile([1, 1], F32, name=f"amax_{sfx}")
            nc.vector.tensor_tensor(amax[:], gm, hmx[:], op=ALU.max)
            ra = small.tile([1, 1], F32, name=f"ra_{sfx}")
            nc.vector.reciprocal(ra[:], amax[:])
            sf = small.tile([1, 1], F32, name=f"sf_{sfx}")
            nc.vector.tensor_scalar_mul(sf[:], ra[:], E4M3_MAX)
            s = small.tile([1, 1], F32, name=f"s_{sfx}")
            nc.vector.reciprocal(s[:], sf[:])
            rh = small.tile([1, 1], F32, name=f"rh_{sfx}")
            nc.vector.tensor_scalar_mul(rh[:], sf[:], 0.5)
            return s, rh

        qi = 0

        def x_amax(i):
            nonlocal qi
            xt = lfp.tile([P, d_tiles, MS], F16, name="lf")
            queues[qi % 2].dma_start(xt[:], x16l[i])
            qi += 1
            nc.vector.reduce_max(
                xacc[:, i : i + 1], xt[:], axis=AX.XY,
                apply_absolute_value=True,
            )

        def w_amax(k):
            nonlocal qi
            if k < WPARK:
                wt = wp_tiles[k]
            else:
                wt = wsr.tile([P, 2, f_shard], F16, name="wtr")
            queues[qi % 2].dma_start(wt[:], w16[k])
            qi += 1
            nc.vector.reduce_max(
                wacc[:, k : k + 1], wt[:], axis=AX.XY,
                apply_absolute_value=True,
            )

        # Interleave 2 x-tiles : 1 w-tile so the x amax finishes ~2/3 in
        # while the combined DMA arrival rate keeps the DVE saturated.
        # AR_x launches mid-stream and its latency hides behind the
        # remaining w reduces.
        seq = []
        xi = wi = 0
        for step in range(loc_chunks + n_pairs):
            if xi < loc_chunks and step % 3 != 2:
                seq.append(("x", xi)); xi += 1
            else:
                seq.append(("w", wi)); wi += 1
        x_done_at = max(i for i, s in enumerate(seq) if s[0] == "x")
        mpr_x = g_x = None
        for idx, (kind, j) in enumerate(seq):
            (x_amax if kind == "x" else w_amax)(j)
            if idx == x_done_at:
                mpr_x = amax_bounce(xacc, "x")
            elif idx == x_done_at + 2:
                g_x = amax_allreduce(mpr_x, "x")

        # AR_w enters the cc stream first (independent of AR_x); the x
        # scale chain + chunk-0/1 quantization hide inside its latency.
        g_w = amax_allreduce(amax_bounce(wacc, "w"), "w")

        s_x, rh_x = scales(g_x[:], histx, "x")
        rhx_b = small.tile([P, 1], F32)
        nc.gpsimd.partition_broadcast(rhx_b[:], rh_x[:])

        # lhsT chunk loader: 1 MB contiguous fp16 DMA + quantize on ScalarE.
        # splits > 1 chop the quant so the first matmuls start sooner.
        def load_chunk(q, splits=1):
            lf = lfp.tile([P, d_tiles, MS], F16, name="lf")
            nc.sync.dma_start(lf[:], xt16[q])
            ql = qlp.tile([P, d_tiles, MS], FP8, name="ql")
            step = d_tiles // splits
            for j in range(splits):
                nc.scalar.mul(
                    ql[:, j * step : (j + 1) * step, :],
                    lf[:, j * step : (j + 1) * step, :],
                    rhx_b[:],
                )
            return ql

        pre = {q: load_chunk(q, splits=4) for q in (0, 1)}

        s_w, rh_w = scales(g_w[:], histw, "w")
        rhw_b = small.tile([P, 1], F32)
        nc.gpsimd.partition_broadcast(rhw_b[:], rh_w[:])

        Cs = small.tile([1, 1], F32)
        nc.vector.tensor_tensor(Cs[:], s_x[:], s_w[:], op=ALU.mult)
        nc.vector.tensor_scalar_mul(Cs[:], Cs[:], 4.0)
        rC = small.tile([1, 1], F32)
        nc.vector.reciprocal(rC[:], Cs[:])
        C_b = small.tile([P, 1], F32)
        nc.gpsimd.partition_broadcast(C_b[:], Cs[:])

        # bias: fp32 -> bf16 -> fp32, then pre-divide by C, broadcast to 128 parts
        btmp = small.tile([1, f_shard], F32)
        nc.gpsimd.dma_start(btmp[:], b_sh[:])
        bbf = small.tile([1, f_shard], BF16)
        nc.vector.tensor_copy(bbf[:], btmp[:])
        nc.vector.tensor_copy(btmp[:], bbf[:])
        nc.vector.tensor_scalar_mul(btmp[:], btmp[:], rC[:])
        bP = small.tile([P, f_shard], F32)
        nc.gpsimd.partition_broadcast(bP[:], btmp[:])

        # ---- phase 3: quantize w (parked pairs instantly, rest re-stream) ----
        # Quant muls alternate ScalarE/VectorE so the matmul k-loop chases
        # at ~1.5us/pair instead of 3us.
        qw_tiles = [
            qwp.tile([P, 2, f_shard], FP8, name=f"qwt{k}") for k in range(n_pairs)
        ]
        for k in range(n_pairs):
            if k < WPARK:
                wt = wp_tiles[k]
            else:
                wt = wsr.tile([P, 2, f_shard], F16, name="wtr")
                (nc.scalar if k % 2 == 0 else nc.gpsimd).dma_start(wt[:], w16[k])
            if k % 2 == 0:
                nc.scalar.mul(qw_tiles[k][:], wt[:], rhw_b[:])
            else:
                nc.vector.tensor_scalar_mul(qw_tiles[k][:], wt[:], rhw_b[:])

        # ---- phase 4: matmul + epilogue ----
        # out[tok, f] = gelu(C * (sum_d qxT[d, tok] * qw[d, f] + bias/C))
        for q in range(n_chunks):
            ql = pre.pop(q) if q in pre else load_chunk(q)
            pss = [
                psum.tile([P, NF], F32, tag="ps", name=f"mmps{n}")
                for n in range(n_tiles)
            ]
            for k in range(n_pairs):
                for n in range(n_tiles):
                    nc.tensor.matmul(
                        pss[n][:],
                        lhsT=ql[:, 2 * k : 2 * k + 2, :],
                        rhs=qw_tiles[k][:, :, n * NF : (n + 1) * NF],
                        start=(k == 0),
                        stop=(k == n_pairs - 1),
                        perf_mode=DR,
                    )
            row = q * MS
            for n in range(n_tiles):
                t1 = ostg.tile([P, NF], F32, name="t1")
                nc.vector.tensor_tensor(
                    t1[:], pss[n][:], bP[:, n * NF : (n + 1) * NF], op=ALU.add
                )
                ot = ostg.tile([P, NF], F32, name="ot")
                nc.scalar.activation(
                    ot[:], t1[:], act_fn, bias=zbias[:], scale=C_b[:]
                )
                oq = nc.gpsimd if n % 2 == 0 else nc.sync
                oq.dma_start(
                    out_sh[row : row + P, n * NF : (n + 1) * NF], ot[:]
                )

    nc.compile()
    return nc


_CACHE = {}


def _get_program(d=D, f_shard=F // NCORES, t_total=T, n_cores=NCORES):
    key = (d, f_shard, t_total, n_cores)
    if key not in _CACHE:
        _CACHE[key] = build_program(d, f_shard, t_total, n_cores)
    return _CACHE[key]


def make_in_maps(x, w, bias, in_hist, k_hist, n_cores=NCORES):
    t_total = x.shape[0]
    d = x.shape[1]
    f_shard = w.shape[1] // n_cores
    d_tiles = d // P
    n_pairs = d_tiles // 2
    n_chunks = t_total // MS
    loc_chunks = n_chunks // n_cores

    # chunk-major fp16 x^T: L[q, p, s, m] = x[q*MS + m, s*P + p]
    x16 = x.astype(np.float16).reshape(n_chunks, MS, d_tiles, P)
    xt16 = np.ascontiguousarray(x16.transpose(0, 3, 2, 1))  # [64, 128, 32, 128]

    ih = np.asarray(in_hist, np.float32).reshape(1, HL)
    kh = np.asarray(k_hist, np.float32).reshape(1, HL)
    in_maps = []
    for r in range(n_cores):
        # pair-major fp16 W: w16[k, p, o, f] = w[(2k+o)*P + p, f]
        w16 = np.ascontiguousarray(
            w[:, r * f_shard : (r + 1) * f_shard]
            .astype(np.float16)
            .reshape(n_pairs, 2, P, f_shard)
            .transpose(0, 2, 1, 3)
        )
        in_maps.append(
            {
                "xt16": xt16,
                "x16_loc": np.ascontiguousarray(
                    xt16[r * loc_chunks : (r + 1) * loc_chunks]
                ),
                "w16": w16,
                "bias_shard": np.ascontiguousarray(
                    bias[r * f_shard : (r + 1) * f_shard], dtype=np.float32
                ).reshape(1, f_shard),
                "in_hist": ih,
                "k_hist": kh,
            }
        )
    return in_maps


def _install_ntff_shim():
    """Provide antenv.axon_hooks (absent in this image) so bass_utils can
    NTFF-profile under axon, wiring it to libaxon_pjrt's nrt profile API."""
    import sys as _sys
    import types

    if "antenv.axon_hooks" in _sys.modules:
        return
    mod = types.ModuleType("antenv.axon_hooks")
    _state = {"hook": None}
    mod.set_axon_ntff_profile_hook = lambda h: _state.__setitem__("hook", h)
    mod.get_axon_ntff_profile_hook = lambda: _state["hook"]
    _sys.modules["antenv.axon_hooks"] = mod
    import antenv

    antenv.axon_hooks = mod
    try:
        from trn_agent_boot.trn_boot import _ntff_profile_via_ctypes

        mod.set_axon_ntff_profile_hook(
            _ntff_profile_via_ctypes("/opt/axon/libaxon_pjrt.so")
        )
    except Exception as e:
        print(f"ntff shim: hook unavailable ({e}); tracing will degrade")


def run(inputs_dict, trace=False, **kw):
    """Run on the 8 NeuronCores; returns (full_output, BassKernelResults)."""
    if trace:
        _install_ntff_shim()
    x = np.asarray(inputs_dict["inputs"], np.float32).reshape(T, D)
    w = np.asarray(inputs_dict["kernel"], np.float32)
    bias = np.asarray(inputs_dict["bias"], np.float32)
    nc = _get_program()
    in_maps = make_in_maps(
        x, w, bias, inputs_dict["input_amax_history"], inputs_dict["kernel_amax_history"]
    )
    old_m = nc.m
    nc.m = get_hw_module(nc.m)
    try:
        res = run_bass_kernel_spmd(
            nc, in_maps, core_ids=list(range(NCORES)), trace=trace, **kw
        )
    finally:
        nc.m = old_m
    out = np.concatenate(
        [res.results[r]["out_shard"] for r in range(NCORES)], axis=1
    )
    return out.reshape(B, S, F).astype(np.float32), res


def kernel(**inputs):
    out, _ = run(inputs, trace=False)
    return out
